# revision 16
# baseline (speedup 1.0000x reference)
"""Disentangled self-attention (DeBERTa-style) TRN2 Bass kernel, v3.

Sharding: tensor-parallel over heads. 8 cores x 2 heads each (H=16).
Each core computes q/k/v and pos projections for its 128 output dims
(2 heads x 64), full attention for its heads over all 4 batches.

Math (per head h, batch b), everything kept UN-normalized on device:
  S[m,n] = q[n].k[m] + q[n].pos_k[d(n,m)] + k[m].pos_q[d(n,m)]
  d(n,m) = clip(n-m+512, 0, 1023);   P = exp(S / SCALE)
  out rows 0..63 = v^T @ P (unnormalized ctx, transposed), row 64 =
  column sums of P (softmax denominators). Host divides + transposes.

Both relative-position biases are sheared (per-row sliding window)
gathers of matmul results, staged to DRAM fp16 and read back skewed:
  A'[n, j] = q[n] . pos_k[clip(1535-j)]  -> read back TRANSPOSED per
      m-block via the DMA XBAR: c2pT[m,n] tile directly (no PE
      transposes).
  B'[m, j] = k[m] . pos_q[clip(j-511)]   -> plain skewed read p2cT[m,n].
Only the live 1152-wide band per 128-row block is staged.

The bias sum (c2pT + p2cT) is written to PSUM by the DVE, and the QK
matmul accumulates on top (start=False), so scores need one DVE op per
block. exp's activation `scale` applies the 1/sqrt(3d) normalization.
"""
import os
import sys

sys.path.insert(0, "/opt/trn_rl_repo")

import numpy as np

import concourse.bacc as bacc
import concourse.bass as bass
import concourse.mybir as mybir
import concourse.tile as tile
from concourse.bass_utils import run_bass_kernel_spmd
from concourse.masks import make_identity

F32 = mybir.dt.float32
F16 = mybir.dt.float16
AX = mybir.AluOpType
EXP = mybir.ActivationFunctionType.Exp

B, N, D, H = 4, 1024, 1024, 16
HD = D // H          # 64
SPAN = 512
SCALE = float(np.sqrt(HD * 3))
NCORES = 8
OL = 128             # output dims per core (2 heads x 64)
JW = 2048            # staging row width
SK = JW - 1          # 2047, skew stride
BAND = 1152          # staged columns per 128-row block

_nc_cache = [None]


def _build_nc():
    nc = bacc.Bacc(None, target_bir_lowering=False, debug=False)

    xT = nc.declare_dram_parameter("xT", [B, D, N], F16, isOutput=False)
    wq = nc.declare_dram_parameter("wq", [D, OL], F16, isOutput=False)
    wk = nc.declare_dram_parameter("wk", [D, OL], F16, isOutput=False)
    wv = nc.declare_dram_parameter("wv", [D, OL], F16, isOutput=False)
    wpk = nc.declare_dram_parameter("wpk", [D, OL], F16, isOutput=False)
    wpq = nc.declare_dram_parameter("wpq", [D, OL], F16, isOutput=False)
    relA = nc.declare_dram_parameter("relA", [D, JW], F16, isOutput=False)
    relB = nc.declare_dram_parameter("relB", [D, JW], F16, isOutput=False)
    out = nc.declare_dram_parameter("out", [2 * B, 65, N], F16,
                                    isOutput=True)

    A16 = [nc.dram_tensor(f"A16_{u}", [N, JW], F16) for u in range(2 * B)]
    B16 = [nc.dram_tensor(f"B16_{u}", [N, JW], F16) for u in range(2 * B)]

    with tile.TileContext(nc) as tc:
        _emit(nc, tc, xT, wq, wk, wv, wpk, wpq, relA, relB, out, A16, B16)
    nc.compile()
    return nc


def _emit(nc, tc, xT, wq, wk, wv, wpk, wpq, relA, relB, out, A16, B16):
    from contextlib import ExitStack
    with ExitStack() as ctx:
        const = ctx.enter_context(tc.tile_pool(name="const", bufs=1))
        relp = ctx.enter_context(tc.tile_pool(name="relp", bufs=4))
        xp = ctx.enter_context(tc.tile_pool(name="xp", bufs=2))
        qkp = ctx.enter_context(tc.tile_pool(name="qkp", bufs=2))
        vp = ctx.enter_context(tc.tile_pool(name="vp", bufs=2))
        v8p = ctx.enter_context(tc.tile_pool(name="v8p", bufs=3))
        stp = ctx.enter_context(tc.tile_pool(name="stp", bufs=3))
        cp = ctx.enter_context(tc.tile_pool(name="cp", bufs=3))
        p16p = ctx.enter_context(tc.tile_pool(name="p16p", bufs=3))
        finp = ctx.enter_context(tc.tile_pool(name="finp", bufs=2))
        ps = ctx.enter_context(tc.tile_pool(name="ps", bufs=4, space="PSUM"))
        vtpp = ctx.enter_context(tc.tile_pool(name="vtpp", bufs=1,
                                              space="PSUM"))
        pvp = ctx.enter_context(tc.tile_pool(name="pvp", bufs=3,
                                             space="PSUM"))

        identf = const.tile([128, 128], F32, tag="identf")
        make_identity(nc, identf[:])
        ident16 = const.tile([128, 128], F16, tag="ident16")
        nc.vector.tensor_copy(ident16[:], identf[:])

        # ---- weights to SBUF: [128(i_sub), 8(i_tile), 128(o)]
        w_sb = {}
        for name, dram in [("wq", wq), ("wk", wk), ("wv", wv),
                           ("wpk", wpk), ("wpq", wpq)]:
            t = const.tile([128, 8, 128], F16, tag=f"w_{name}")
            nc.sync.dma_start(t[:], dram[:].rearrange("(t p) o -> p t o",
                                                      p=128))
            w_sb[name] = t

        # ---- pos tables (unnormalized): posA[o,j] = pos_k[clip(1535-j)][o]
        #      posB[o,j] = pos_q[clip(j-511)][o]
        pos16 = {}
        for tbl, (rel_dram, wname) in (("A", (relA, "wpk")),
                                       ("B", (relB, "wpq"))):
            pt = const.tile([128, JW], F16, tag=f"pos{tbl}")
            for jc in range(4):
                acc = ps.tile([128, 512], F32, tag="p512")
                for it in range(8):
                    rt = relp.tile([128, 512], F16, tag="relt")
                    nc.sync.dma_start(
                        rt[:], rel_dram[it * 128:(it + 1) * 128,
                                        jc * 512:(jc + 1) * 512])
                    nc.tensor.matmul(acc[:], w_sb[wname][:, it, :], rt[:],
                                     start=(it == 0), stop=(it == 7))
                nc.scalar.copy(pt[:, jc * 512:(jc + 1) * 512], acc[:])
            pos16[tbl] = pt

        for b in range(B):
            x16t = xp.tile([128, 8, N], F16, tag="x16")
            nc.sync.dma_start(x16t[:], xT[b].rearrange("(t p) n -> p t n",
                                                       p=128))

            # ---- projections (all fp16)
            q16 = qkp.tile([128, N], F16, tag="q16")
            k16 = qkp.tile([128, N], F16, tag="k16")
            v16 = vp.tile([128, N], F16, tag="v16")
            for wname, dst, eng in (("wq", q16, "s"), ("wk", k16, "v"),
                                    ("wv", v16, "s")):
                for nh in range(2):
                    acc = ps.tile([128, 512], F32, tag="p512")
                    for it in range(8):
                        nc.tensor.matmul(
                            acc[:], w_sb[wname][:, it, :],
                            x16t[:, it, nh * 512:(nh + 1) * 512],
                            start=(it == 0), stop=(it == 7))
                    if eng == "s":
                        nc.scalar.copy(dst[:, nh * 512:(nh + 1) * 512],
                                       acc[:])
                    else:
                        nc.vector.tensor_copy(
                            dst[:, nh * 512:(nh + 1) * 512], acc[:])

            # ---- v transposed to [n-part, mb, o(64)+ones] per head
            v16u = [v8p.tile([128, 8, 65], F16, tag="v16u", name=f"v16u{h}")
                    for h in range(2)]
            for h in range(2):
                nc.gpsimd.memset(v16u[h][:, :, 64:65], 1.0)
            for nb in range(8):
                tp = vtpp.tile([128, 128], F16, tag="vtp")
                nc.tensor.transpose(tp[:], v16[:, nb * 128:(nb + 1) * 128],
                                    ident16[:])
                for h in range(2):
                    nc.vector.tensor_copy(v16u[h][:, nb, 0:64],
                                          tp[:, h * 64:h * 64 + 64])

            for h in range(2):
                u = b * 2 + h
                h0 = h * 64
                qh = q16[h0:h0 + 64, :]
                kh = k16[h0:h0 + 64, :]

                # ---- staging A' and B' (fp16), live 1152-band only
                for tbl, src, dstd in (("A", qh, A16[u]),
                                       ("B", kh, B16[u])):
                    pos_t = pos16[tbl][h0:h0 + 64, :]
                    for nb in range(8):
                        c0 = (7 - nb) * 128
                        st = stp.tile([128, BAND], F16, tag="st")
                        for off, w in ((0, 512), (512, 512), (1024, 128)):
                            acc = ps.tile([128, 512], F32, tag="p512")
                            nc.tensor.matmul(
                                acc[:, 0:w],
                                src[:, nb * 128:(nb + 1) * 128],
                                pos_t[:, c0 + off:c0 + off + w],
                                start=True, stop=True)
                            if (nb + (0 if tbl == "A" else 1)) % 2 == 0:
                                nc.scalar.copy(st[:, off:off + w],
                                               acc[:, 0:w])
                            else:
                                nc.vector.tensor_copy(st[:, off:off + w],
                                                      acc[:, 0:w])
                        nc.sync.dma_start(
                            dstd[nb * 128:(nb + 1) * 128, c0:c0 + BAND],
                            st[:])

                # ---- scores (transposed [m, n]), exp fp16, PV fp16
                pv = [pvp.tile([65, 512], F32, tag="pv", name=f"pv{i}")
                      for i in range(2)]
                for mb in range(8):
                    m0 = mb * 128
                    c2pT = cp.tile([128, N], F16, tag="c2pT")
                    nc.sync.dma_start(
                        c2pT[:], bass.AP(tensor=A16[u], offset=1023 + m0,
                                         ap=[[SK, N], [1, 128]]),
                        transpose=True)
                    p2cT = cp.tile([128, N], F16, tag="p2cT")
                    nc.sync.dma_start(
                        p2cT[:], bass.AP(tensor=B16[u],
                                         offset=m0 * SK + 1023,
                                         ap=[[SK, 128], [1, N]]))
                    P16t = p16p.tile([128, N], F16, tag="P16")
                    for nh in range(2):
                        n0 = nh * 512
                        b16 = cp.tile([128, 512], F16, tag="b16")
                        nc.gpsimd.tensor_tensor(
                            b16[:], c2pT[:, n0:n0 + 512],
                            p2cT[:, n0:n0 + 512], AX.add)
                        S = ps.tile([128, 512], F32, tag="p512")
                        nc.tensor.matmul(
                            S[:], kh[:, m0:m0 + 128], qh[:, n0:n0 + 512],
                            start=True, stop=True)
                        nc.vector.scalar_tensor_tensor(
                            S[:], S[:], 1.0, b16[:], op0=AX.mult,
                            op1=AX.add)
                        nc.scalar.activation(P16t[:, n0:n0 + 512], S[:],
                                             EXP, scale=1.0 / SCALE)
                    for nh in range(2):
                        nc.tensor.matmul(
                            pv[nh][:], v16u[h][:, mb, :],
                            P16t[:, nh * 512:(nh + 1) * 512],
                            start=(mb == 0), stop=(mb == 7))

                # ---- evac pv [65, 1024] fp16; host divides + transposes
                ctxo = finp.tile([65, N], F16, tag="ctxo")
                for nh in range(2):
                    nc.vector.tensor_copy(ctxo[:, nh * 512:(nh + 1) * 512],
                                          pv[nh][:])
                nc.sync.dma_start(out[u], ctxo[:])


def _prep_in_maps(inputs):
    x = np.ascontiguousarray(np.asarray(inputs["hidden_states"], np.float32))
    re = np.asarray(inputs["rel_embeddings"], np.float32)
    Wq = np.asarray(inputs["Wq"], np.float32)
    Wk = np.asarray(inputs["Wk"], np.float32)
    Wv = np.asarray(inputs["Wv"], np.float32)
    Wpk = np.asarray(inputs["Wpk"], np.float32)
    Wpq = np.asarray(inputs["Wpq"], np.float32)

    xTh = np.ascontiguousarray(x.transpose(0, 2, 1)).astype(np.float16)
    jA = np.clip(1535 - np.arange(JW), 0, 2 * SPAN - 1)
    relAh = np.ascontiguousarray(re[jA].T).astype(np.float16)
    jB = np.clip(np.arange(JW) - 511, 0, 2 * SPAN - 1)
    relBh = np.ascontiguousarray(re[jB].T).astype(np.float16)

    in_maps = []
    for c in range(NCORES):
        sl = slice(OL * c, OL * (c + 1))
        in_maps.append(dict(
            xT=xTh, relA=relAh, relB=relBh,
            wq=np.ascontiguousarray(Wq[sl].T).astype(np.float16),
            wk=np.ascontiguousarray(Wk[sl].T).astype(np.float16),
            wv=np.ascontiguousarray(Wv[sl].T).astype(np.float16),
            wpk=np.ascontiguousarray(Wpk[sl].T).astype(np.float16),
            wpq=np.ascontiguousarray(Wpq[sl].T).astype(np.float16),
        ))
    return in_maps


def _run(inputs, **kw):
    in_maps = _prep_in_maps(inputs)
    if _nc_cache[0] is None:
        _nc_cache[0] = _build_nc()
    return run_bass_kernel_spmd(_nc_cache[0], in_maps, list(range(NCORES)),
                                **kw)


def _assemble(res):
    full = np.empty((B, N, D), np.float32)
    for c in range(NCORES):
        o = np.asarray(res.results[c]["out"], np.float32)  # [2B, 65, N]
        ctx = o[:, 0:64, :] / o[:, 64:65, :]
        for u in range(2 * B):
            b, h = divmod(u, 2)
            full[b, :, c * OL + h * 64:c * OL + (h + 1) * 64] = ctx[u].T
    return full


def kernel(**inputs):
    return _assemble(_run(inputs))


def run_profiled(**inputs):
    return _run(inputs, trace=True)
